# revision 15
# baseline (speedup 1.0000x reference)
"""Trainium2 Bass kernel for nn_Block_54219667145535 (linear-attention block).

fp8 (e4m3) DoubleRow matmuls throughout; weights SBUF-resident; fused
attention+FFN pipeline per 512-token group. 8 cores, 2 per batch (B=4):
each core computes k/v + [D,D] kv state on its own 2048 tokens and
pair-AllReduces (bf16) before attention. Precision recovery: x1 and the
silu-product are fed to the FFN matmuls as two-term fp8 (hi + lo).
"""

import os
import sys
from contextlib import ExitStack

import numpy as np


def _ensure_paths():
    for p in ("/opt/trn_rl_repo", "/root/.axon_site/_ro/trn_rl_repo"):
        if os.path.isdir(p) and p not in sys.path:
            sys.path.insert(0, p)
    try:
        import concourse.bass  # noqa: F401
    except ImportError as e:  # pragma: no cover
        raise ImportError(f"concourse not importable: {e}")


_ensure_paths()

import ml_dtypes  # noqa: E402

import concourse.bass as bass  # noqa: E402
import concourse.bacc as bacc  # noqa: E402
import concourse.tile as tile  # noqa: E402
from concourse import mybir  # noqa: E402
from concourse.masks import make_identity  # noqa: E402

F32 = mybir.dt.float32
BF16 = mybir.dt.bfloat16
F8 = mybir.dt.float8e4
AF = mybir.ActivationFunctionType
ALU = mybir.AluOpType
DR = mybir.MatmulPerfMode.DoubleRow

NP_F8 = ml_dtypes.float8_e4m3  # TRN-compatible e4m3 (max 240)
NP_BF = ml_dtypes.bfloat16

D = 1024
DCH = 8        # d chunks of 128
H_PAD = 2816
HCH = 22       # h chunks of 128
LN_EPS = 1e-5
ATTN_EPS = 1e-6
WS = 64.0      # weight pre-scale (host multiplies W by WS before fp8 cast)
INV = 1.0 / WS
EKV = D + 8    # kv e-columns + 8 extra (col D = ksum via ones-column of v)

X1_2T = True    # two-term fp8 x1 into gate/up
PROD_2T = True  # two-term fp8 silu-product into down


def ts(i, sz):
    return bass.ts(i, sz)


def _bcast_row(row_ap, parts=128):
    """AP reading a [1, N] DRAM row replicated across `parts` partitions."""
    return bass.AP(
        tensor=row_ap.tensor,
        offset=row_ap.offset,
        ap=[[0, parts]] + [list(d) for d in row_ap.ap[1:]],
    )


def build_program(T_OWN=2048, n_cores=8):
    NBLK = T_OWN // 512   # 512-token blocks in P1
    NT4 = T_OWN // 128    # 128-token groups
    NTG = T_OWN // 512    # 512-token groups in P2
    GROUPS = [[c, c + 1] for c in range(0, n_cores, 2)]

    nc = bacc.Bacc(
        "TRN2",
        target_bir_lowering=False,
        debug=False,
        enable_asserts=False,
        num_devices=n_cores,
        num_swdge_queues=4,
    )

    # ---- I/O ----
    xT8 = nc.dram_tensor("xT8", [128, T_OWN // 512, DCH, 512], F8,
                     kind="ExternalInput").ap()
    xtok = nc.dram_tensor("xtok", [T_OWN, D], BF16, kind="ExternalInput").ap()
    wq8 = nc.dram_tensor("wq8", [128, DCH, D], F8, kind="ExternalInput").ap()
    wk8 = nc.dram_tensor("wk8", [128, DCH, D], F8, kind="ExternalInput").ap()
    wv8 = nc.dram_tensor("wv8", [128, DCH, D], F8, kind="ExternalInput").ap()
    wg8 = nc.dram_tensor("wg8", [128, DCH, H_PAD], F8, kind="ExternalInput").ap()
    wu8 = nc.dram_tensor("wu8", [128, DCH, H_PAD], F8, kind="ExternalInput").ap()
    wd8 = nc.dram_tensor("wd8", [128, HCH, D], F8, kind="ExternalInput").ap()
    bq_col = nc.dram_tensor("bq_col", [128, DCH], F32, kind="ExternalInput").ap()
    nbq_col = nc.dram_tensor("nbq_col", [128, DCH], F32, kind="ExternalInput").ap()
    bg_col = nc.dram_tensor("bg_col", [128, HCH], F32, kind="ExternalInput").ap()
    bu_col = nc.dram_tensor("bu_col", [128, HCH], F32, kind="ExternalInput").ap()
    bk_row = nc.dram_tensor("bk_row", [1, D], BF16, kind="ExternalInput").ap()
    bv_row = nc.dram_tensor("bv_row", [1, D], BF16, kind="ExternalInput").ap()
    g1_row = nc.dram_tensor("g1_row", [1, D], BF16, kind="ExternalInput").ap()
    b1_row = nc.dram_tensor("b1_row", [1, D], BF16, kind="ExternalInput").ap()
    g2_row = nc.dram_tensor("g2_row", [1, D], BF16, kind="ExternalInput").ap()
    b2_row = nc.dram_tensor("b2_row", [1, D], BF16, kind="ExternalInput").ap()
    bd_row = nc.dram_tensor("bd_row", [1, D], BF16, kind="ExternalInput").ap()
    out = nc.dram_tensor("out", [T_OWN, D], F32, kind="ExternalOutput").ap()

    input_names = [
        "xT8", "xtok", "wq8", "wk8", "wv8", "wg8", "wu8", "wd8",
        "bq_col", "nbq_col", "bg_col", "bu_col", "bk_row", "bv_row",
        "g1_row", "b1_row", "g2_row", "b2_row", "bd_row",
    ]

    with tile.TileContext(nc) as tc, ExitStack() as top:
        dram = top.enter_context(tc.tile_pool(name="dram", bufs=1, space="DRAM"))
        kvH_ci = [dram.tile([128, DCH // 2, EKV], BF16, name=f"kvH{i}_ci")
                  for i in range(2)]
        kvH_co = [dram.tile([128, DCH // 2, EKV], BF16, name=f"kvH{i}_co")
                  for i in range(2)]

        # pools first, DMA kicks emitted in priority order below
        consts = top.enter_context(tc.tile_pool(name="consts", bufs=1))
        ident_b = consts.tile([128, 128], BF16, name="ident_b")
        epsb = consts.tile([128, 1], F32, name="epsb")
        bq_s = consts.tile([128, DCH], F32, name="bq_s")
        nbq_s = consts.tile([128, DCH], F32, name="nbq_s")
        bg_s = consts.tile([128, HCH], F32, name="bg_s")
        bu_s = consts.tile([128, HCH], F32, name="bu_s")
        bkb = consts.tile([128, D], BF16, name="bkb")
        bvb = consts.tile([128, D], BF16, name="bvb")
        g1b = consts.tile([128, D], BF16, name="g1b")
        b1b = consts.tile([128, D], BF16, name="b1b")
        g2b = consts.tile([128, D], BF16, name="g2b")
        b2b = consts.tile([128, D], BF16, name="b2b")
        bdb = consts.tile([128, D], BF16, name="bdb")

        # FFN weights + long-lived activations (whole-kernel lifetime)
        wpool = top.enter_context(tc.tile_pool(name="wpool", bufs=1))
        wg_s = wpool.tile([128, DCH, H_PAD], F8, name="wg_s")
        wu_s = wpool.tile([128, DCH, H_PAD], F8, name="wu_s")
        wd_s = wpool.tile([128, HCH, D], F8, name="wd_s")
        qp8 = wpool.tile([128, DCH, T_OWN], F8, name="qp8")
        kv8 = wpool.tile([128, DCH, EKV], F8, name="kv8")

        # kv collective staging (SBUF side); closes after the post-collective
        # casts, so it must be opened before the xw pool (LIFO release order)
        kvsb_sc = ExitStack()
        kvsb_p = kvsb_sc.enter_context(tc.tile_pool(name="kvsb", bufs=1))
        kv_sb = kvsb_p.tile([128, DCH, EKV], BF16, name="kv_sb")

        # qkv weights + x / kp / v: freed after the q projection
        xw_sc = ExitStack()
        xw_p = xw_sc.enter_context(tc.tile_pool(name="xw", bufs=1))
        wk_s = xw_p.tile([128, DCH, D], F8, name="wk_s")
        wv_s = xw_p.tile([128, DCH, D], F8, name="wv_s")
        wq_s = xw_p.tile([128, DCH, D], F8, name="wq_s")
        xT8_s = xw_p.tile([128, NBLK, DCH, 512], F8, name="xT8_s")
        kp8 = xw_p.tile([128, NT4, D], F8, name="kp8")
        v8 = xw_p.tile([128, NT4, EKV], F8, name="v8")

        # --- DMA kicks, critical path first -------------------------------
        # scalar ring: k/v weights (P1-critical), then the rest
        nc.scalar.dma_start(out=wk_s[:], in_=wk8)
        nc.scalar.dma_start(out=wv_s[:], in_=wv8)
        nc.scalar.dma_start(out=bkb[:], in_=_bcast_row(bk_row))
        nc.scalar.dma_start(out=bvb[:], in_=_bcast_row(bv_row))
        nc.scalar.dma_start(out=bq_s[:], in_=bq_col)
        nc.scalar.dma_start(out=nbq_s[:], in_=nbq_col)
        nc.scalar.dma_start(out=wq_s[:], in_=wq8)
        nc.scalar.dma_start(out=bg_s[:], in_=bg_col)
        nc.scalar.dma_start(out=bu_s[:], in_=bu_col)
        nc.scalar.dma_start(out=g1b[:], in_=_bcast_row(g1_row))
        nc.scalar.dma_start(out=b1b[:], in_=_bcast_row(b1_row))
        nc.scalar.dma_start(out=g2b[:], in_=_bcast_row(g2_row))
        nc.scalar.dma_start(out=b2b[:], in_=_bcast_row(b2_row))
        nc.scalar.dma_start(out=bdb[:], in_=_bcast_row(bd_row))
        # device-generated consts
        make_identity(nc, ident_b[:])
        nc.vector.memset(epsb[:], LN_EPS)
        nc.vector.memset(v8[:, :, D:D + 1], 1.0)
        nc.vector.memset(v8[:, :, D + 1:EKV], 0.0)

        # ---------------- P1: k/v projection over own tokens --------------
        with ExitStack() as p1:
            t1_p = p1.enter_context(tc.tile_pool(name="p1tmp", bufs=4))
            ps1 = p1.enter_context(
                tc.tile_pool(name="ps1", bufs=8, space="PSUM"))
            for blk in range(NBLK):
                nc.sync.dma_start(out=xT8_s[:, blk], in_=xT8[:, blk])
                if blk == 1:
                    # FFN weights: kicked once P1 is underway so they don't
                    # compete with the P1-critical loads
                    nc.gpsimd.dma_start(out=wg_s[:], in_=wg8)
                    nc.gpsimd.dma_start(out=wu_s[:], in_=wu8)
                    nc.gpsimd.dma_start(out=wd_s[:], in_=wd8)
                for t4p in range(2):
                    pk = {}
                    pv = {}
                    for li in range(2):
                        t4 = blk * 4 + t4p * 2 + li
                        for h in range(2):
                            pk[(li, h)] = ps1.tile(
                                [128, 512], F32, name=f"pk{t4}_{h}", tag="ps1")
                            pv[(li, h)] = ps1.tile(
                                [128, 512], F32, name=f"pv{t4}_{h}", tag="ps1")
                    for j in range(DCH // 2):
                        for li in range(2):
                            t4 = blk * 4 + t4p * 2 + li
                            lt4 = t4p * 2 + li
                            stat = xT8_s[:, blk, 2 * j:2 * j + 2,
                                         ts(lt4, 128)]
                            st = (j == 0)
                            sp = (j == DCH // 2 - 1)
                            for h in range(2):
                                nc.tensor.matmul(
                                    pk[(li, h)][:], stat,
                                    wk_s[:, 2 * j:2 * j + 2, ts(h, 512)],
                                    start=st, stop=sp, perf_mode=DR)
                                nc.tensor.matmul(
                                    pv[(li, h)][:], stat,
                                    wv_s[:, 2 * j:2 * j + 2, ts(h, 512)],
                                    start=st, stop=sp, perf_mode=DR)
                    for li in range(2):
                        t4 = blk * 4 + t4p * 2 + li
                        for h in range(2):
                            hsl = ts(h, 512)
                            # kp = exp(min(kb,0)) + relu(kb), kb = k + bk
                            kb = t1_p.tile([128, 512], BF16,
                                           name=f"kb{t4}_{h}", tag="kb")
                            nc.vector.scalar_tensor_tensor(
                                out=kb[:], in0=pk[(li, h)][:], scalar=INV,
                                in1=bkb[:, hsl], op0=ALU.mult, op1=ALU.add)
                            rl = t1_p.tile([128, 512], BF16,
                                           name=f"rl{t4}_{h}", tag="rl")
                            nc.scalar.activation(rl[:], kb[:], AF.Relu)
                            nc.gpsimd.tensor_tensor(
                                out=kb[:], in0=kb[:], in1=rl[:],
                                op=ALU.subtract)
                            nc.scalar.activation(kb[:], kb[:], AF.Exp)
                            nc.gpsimd.tensor_tensor(
                                out=kp8[:, t4, hsl], in0=kb[:], in1=rl[:],
                                op=ALU.add)
                            nc.vector.scalar_tensor_tensor(
                                out=v8[:, t4, hsl], in0=pv[(li, h)][:],
                                scalar=INV, in1=bvb[:, hsl],
                                op0=ALU.mult, op1=ALU.add)

        # ---------------- kv sweeps (contract over all own tokens) --------
        with ExitStack() as sw:
            ps_sw = sw.enter_context(
                tc.tile_pool(name="ps_sw", bufs=4, space="PSUM"))
            ps_swc = sw.enter_context(
                tc.tile_pool(name="ps_swc", bufs=2, space="PSUM"))
            for dc in range(DCH):
                dsl = ts(dc, 128)
                pa = ps_sw.tile([128, 512], F32, name=f"pkva{dc}", tag="ps_sw")
                pb = ps_sw.tile([128, 512], F32, name=f"pkvb{dc}", tag="ps_sw")
                pc = ps_swc.tile([128, 8], F32, name=f"pkvc{dc}", tag="ps_swc")
                for tp in range(NT4 // 2):
                    stat = kp8[:, 2 * tp:2 * tp + 2, dsl]
                    st = (tp == 0)
                    sp = (tp == NT4 // 2 - 1)
                    nc.tensor.matmul(pa[:], stat,
                                     v8[:, 2 * tp:2 * tp + 2, 0:512],
                                     start=st, stop=sp, perf_mode=DR)
                    nc.tensor.matmul(pb[:], stat,
                                     v8[:, 2 * tp:2 * tp + 2, 512:1024],
                                     start=st, stop=sp, perf_mode=DR)
                    nc.tensor.matmul(pc[:], stat,
                                     v8[:, 2 * tp:2 * tp + 2, D:EKV],
                                     start=st, stop=sp, perf_mode=DR)
                nc.vector.tensor_copy(out=kv_sb[:, dc, 0:512], in_=pa[:])
                nc.vector.tensor_copy(out=kv_sb[:, dc, 512:1024], in_=pb[:])
                nc.vector.tensor_copy(out=kv_sb[:, dc, D:EKV], in_=pc[:])
                if dc == DCH // 2 - 1 or dc == DCH - 1:
                    # pair-AllReduce of this dc-half, overlapped with the
                    # remaining sweeps / q projection
                    hf = 0 if dc < DCH // 2 else 1
                    hs = slice(hf * (DCH // 2), (hf + 1) * (DCH // 2))
                    nc.sync.dma_start(out=kvH_ci[hf][:], in_=kv_sb[:, hs, :])
                    nc.gpsimd.collective_compute(
                        "AllReduce", ALU.add, ins=[kvH_ci[hf][:]],
                        outs=[kvH_co[hf][:]], replica_groups=GROUPS)

        # ---------------- q projection (overlaps the collectives) ---------
        with ExitStack() as qsc:
            tq_p = qsc.enter_context(tc.tile_pool(name="qtmp", bufs=4))
            ps_q = qsc.enter_context(
                tc.tile_pool(name="ps_q", bufs=4, space="PSUM"))
            for qc in range(DCH):
                pq = [ps_q.tile([128, 512], F32, name=f"pq{qc}_{b}", tag="ps_q")
                      for b in range(NBLK)]
                for j in range(DCH // 2):
                    stat = wq_s[:, 2 * j:2 * j + 2, ts(qc, 128)]
                    st = (j == 0)
                    sp = (j == DCH // 2 - 1)
                    for b in range(NBLK):
                        nc.tensor.matmul(
                            pq[b][:], stat,
                            xT8_s[:, b, 2 * j:2 * j + 2, :],
                            start=st, stop=sp, perf_mode=DR)
                for b in range(NBLK):
                    bsl = ts(b, 512)
                    rn = tq_p.tile([128, 512], BF16, name=f"rn{qc}_{b}",
                                   tag="rn")
                    nc.scalar.activation(rn[:], pq[b][:], AF.Relu,
                                         bias=nbq_s[:, qc:qc + 1], scale=-INV)
                    ex = tq_p.tile([128, 512], BF16, name=f"exq{qc}_{b}",
                                   tag="exq")
                    nc.scalar.activation(ex[:], rn[:], AF.Exp, scale=-1.0)
                    qb = tq_p.tile([128, 512], BF16, name=f"qb{qc}_{b}",
                                   tag="qb")
                    nc.vector.tensor_scalar(
                        out=qb[:], in0=pq[b][:], scalar1=INV,
                        scalar2=bq_s[:, qc:qc + 1], op0=ALU.mult, op1=ALU.add)
                    nc.vector.tensor_tensor(out=rn[:], in0=rn[:], in1=ex[:],
                                            op=ALU.add)
                    nc.vector.tensor_tensor(
                        out=qp8[:, qc, bsl], in0=qb[:], in1=rn[:], op=ALU.add)
        xw_sc.close()

        # ---- collective results -> fp8 (kv8 = kv/64, col D = ksum/64) ----
        with kvsb_sc:
            for hf in range(2):
                hs = slice(hf * (DCH // 2), (hf + 1) * (DCH // 2))
                nc.gpsimd.dma_start(out=kv_sb[:, hs, :], in_=kvH_co[hf][:])
                nc.scalar.activation(kv8[:, hs, :], kv_sb[:, hs, :],
                                     AF.Copy, scale=INV)

        # ---------------- P2: attention + LN1 + FFN + LN2, per 512 toks ---
        with ExitStack() as p2:
            xt_p = p2.enter_context(tc.tile_pool(name="xtok_p", bufs=4))
            h1_p = p2.enter_context(tc.tile_pool(name="h1_p", bufs=3))
            h2_p = p2.enter_context(tc.tile_pool(name="h2_p", bufs=2))
            x1_p = p2.enter_context(tc.tile_pool(name="x1_p", bufs=8))
            x1T_p = p2.enter_context(tc.tile_pool(name="x1T_p", bufs=2))
            pr_p = p2.enter_context(tc.tile_pool(name="pr_p", bufs=1))
            st_p = p2.enter_context(tc.tile_pool(name="st_p", bufs=2))
            den_p = p2.enter_context(tc.tile_pool(name="den_p", bufs=2))
            uv_p = p2.enter_context(tc.tile_pool(name="uv_p", bufs=3))
            out_p = p2.enter_context(tc.tile_pool(name="out_p", bufs=2))
            ps_num = p2.enter_context(
                tc.tile_pool(name="ps_num", bufs=2, space="PSUM"))
            ps_den = p2.enter_context(
                tc.tile_pool(name="ps_den", bufs=1, space="PSUM"))
            ps_tr = p2.enter_context(
                tc.tile_pool(name="ps_tr", bufs=1, space="PSUM"))
            ps_gu = p2.enter_context(
                tc.tile_pool(name="ps_gu", bufs=2, space="PSUM"))
            ps_dn = p2.enter_context(
                tc.tile_pool(name="ps_dn", bufs=2, space="PSUM"))

            prod8 = pr_p.tile([128, HCH, 512], F8, name="prod8")
            plo8 = pr_p.tile([128, HCH, 512], F8, name="plo8") if PROD_2T \
                else None

            x1s = {}    # tg -> [x1 tile per t4]
            x1Ts = {}   # tg -> (hi, lo)

            def attn_block(tg):
                """den + num + h1 + LN1 + transpose for one 512-token group."""
                o = tg * 512
                x1_list = []
                x1s[tg] = x1_list
                pdn = ps_den.tile([128, 4], F32, name=f"pdn{tg}", tag="ps_den")
                rden = den_p.tile([128, 4], F32, name=f"rden{tg}", tag="rden")
                x1T_hi = x1T_p.tile([128, DCH, 512], F8,
                                    name=f"x1h{tg}", tag="x1h")
                x1T_lo = x1T_p.tile([128, DCH, 512], F8,
                                    name=f"x1l{tg}", tag="x1l") if X1_2T \
                    else None
                x1Ts[tg] = (x1T_hi, x1T_lo)

                for t4 in range(4):
                    tok = o + t4 * 128
                    xts = xt_p.tile([128, D], BF16, name=f"xt{tg}_{t4}",
                                    tag="xt")
                    nc.sync.dma_start(out=xts[:], in_=xtok[tok:tok + 128, :])
                    h1 = h1_p.tile([128, D], BF16, name=f"h1_{tg}_{t4}",
                                   tag="h1")
                    pn = {}
                    for ec in range(2):
                        pn[ec] = ps_num.tile([128, 512], F32,
                                             name=f"pn{tg}_{t4}_{ec}",
                                             tag="ps_num")
                    for j in range(DCH // 2):
                        stat = qp8[:, 2 * j:2 * j + 2, tok:tok + 128]
                        st = (j == 0)
                        sp = (j == DCH // 2 - 1)
                        nc.tensor.matmul(
                            pdn[:, t4:t4 + 1], stat,
                            kv8[:, 2 * j:2 * j + 2, D:D + 1],
                            start=st, stop=sp, perf_mode=DR)
                        for ec in range(2):
                            nc.tensor.matmul(
                                pn[ec][:], stat,
                                kv8[:, 2 * j:2 * j + 2, ts(ec, 512)],
                                start=st, stop=sp, perf_mode=DR)
                    nc.vector.tensor_scalar_add(
                        out=rden[:, t4:t4 + 1], in0=pdn[:, t4:t4 + 1],
                        scalar1=ATTN_EPS * INV)
                    nc.vector.reciprocal(out=rden[:, t4:t4 + 1],
                                         in_=rden[:, t4:t4 + 1])
                    for ec in range(2):
                        esl = ts(ec, 512)
                        nc.vector.scalar_tensor_tensor(
                            out=h1[:, esl], in0=pn[ec][:],
                            scalar=rden[:, t4:t4 + 1], in1=xts[:, esl],
                            op0=ALU.mult, op1=ALU.add)
                    # LN1
                    stats = st_p.tile([128, 2, 6], F32, name=f"s1_{tg}_{t4}",
                                      tag="st1")
                    nc.vector.bn_stats(out=stats[:, 0, :], in_=h1[:, 0:512])
                    nc.vector.bn_stats(out=stats[:, 1, :], in_=h1[:, 512:1024])
                    mv = st_p.tile([128, 2], F32, name=f"mv1_{tg}_{t4}",
                                   tag="mv1")
                    nc.vector.bn_aggr(out=mv[:], in_=stats[:])
                    rstd = st_p.tile([128, 1], F32, name=f"rs1_{tg}_{t4}",
                                     tag="rstd1")
                    nc.scalar.activation(rstd[:], mv[:, 1:2], AF.Sqrt,
                                         bias=epsb[:])
                    nc.vector.reciprocal(out=rstd[:], in_=rstd[:])
                    xm = st_p.tile([128, D], BF16, name=f"xm_{tg}_{t4}",
                                   tag="xm")
                    nc.vector.tensor_scalar(
                        out=xm[:], in0=h1[:], scalar1=mv[:, 0:1],
                        scalar2=rstd[:], op0=ALU.subtract, op1=ALU.mult)
                    nc.gpsimd.tensor_tensor(out=xm[:], in0=xm[:], in1=g1b[:],
                                            op=ALU.mult)
                    x1 = x1_p.tile([128, D], BF16, name=f"x1_{tg}_{t4}",
                                   tag="x1")
                    nc.vector.tensor_tensor(out=x1[:], in0=xm[:], in1=b1b[:],
                                            op=ALU.add)
                    x1_list.append(x1)

                    # transpose x1 -> d-major; hi/lo fp8 split
                    ptx = ps_tr.tile([128, DCH, 128], BF16,
                                     name=f"ptx{tg}_{t4}", tag="ps_trx")
                    for dc in range(DCH):
                        nc.tensor.transpose(ptx[:, dc, :],
                                            x1[:, ts(dc, 128)], ident_b[:])
                    tsl = ts(t4, 128)
                    nc.scalar.copy(out=x1T_hi[:, :, tsl], in_=ptx[:])
                    if X1_2T:
                        nc.vector.tensor_tensor(
                            out=x1T_lo[:, :, tsl], in0=ptx[:],
                            in1=x1T_hi[:, :, tsl], op=ALU.subtract)

            def gate_up(tg):
                x1T_hi, x1T_lo = x1Ts[tg]
                for hc in range(HCH):
                    pg = ps_gu.tile([128, 512], F32, name=f"pg{tg}_{hc}",
                                    tag="ps_gu")
                    pu = ps_gu.tile([128, 512], F32, name=f"pu{tg}_{hc}",
                                    tag="ps_gu")
                    nterm = 2 if X1_2T else 1
                    for j in range(DCH // 2):
                        sg = wg_s[:, 2 * j:2 * j + 2, ts(hc, 128)]
                        su = wu_s[:, 2 * j:2 * j + 2, ts(hc, 128)]
                        for term in range(nterm):
                            mv_ = x1T_hi if term == 0 else x1T_lo
                            st = (j == 0 and term == 0)
                            sp = (j == DCH // 2 - 1 and term == nterm - 1)
                            nc.tensor.matmul(
                                pg[:], sg, mv_[:, 2 * j:2 * j + 2, :],
                                start=st, stop=sp, perf_mode=DR)
                            nc.tensor.matmul(
                                pu[:], su, mv_[:, 2 * j:2 * j + 2, :],
                                start=st, stop=sp, perf_mode=DR)
                    sig = uv_p.tile([128, 512], BF16, name=f"sig{tg}_{hc}",
                                    tag="sig")
                    nc.scalar.activation(sig[:], pg[:], AF.Silu,
                                         bias=bg_s[:, hc:hc + 1], scale=INV)
                    uv = uv_p.tile([128, 512], BF16, name=f"uv{tg}_{hc}",
                                   tag="uv")
                    nc.vector.tensor_scalar(
                        out=uv[:], in0=pu[:], scalar1=INV,
                        scalar2=bu_s[:, hc:hc + 1], op0=ALU.mult, op1=ALU.add)
                    if PROD_2T:
                        pt = uv_p.tile([128, 512], BF16, name=f"pt{tg}_{hc}",
                                       tag="pt")
                        nc.gpsimd.tensor_tensor(out=pt[:], in0=uv[:],
                                                in1=sig[:], op=ALU.mult)
                        nc.scalar.copy(out=prod8[:, hc, :], in_=pt[:])
                        nc.vector.tensor_tensor(
                            out=plo8[:, hc, :], in0=pt[:],
                            in1=prod8[:, hc, :], op=ALU.subtract)
                    else:
                        nc.gpsimd.tensor_tensor(out=prod8[:, hc, :],
                                                in0=uv[:], in1=sig[:],
                                                op=ALU.mult)

            def down_ln2(tg):
                o = tg * 512
                x1_list = x1s[tg]
                for t4 in range(4):
                    tok = o + t4 * 128
                    tsl = ts(t4, 128)
                    pd = {}
                    for dh in range(2):
                        pd[dh] = ps_dn.tile([128, 512], F32,
                                            name=f"pd{tg}_{t4}_{dh}",
                                            tag="ps_dn")
                    nterm = 2 if PROD_2T else 1
                    for hp in range(HCH // 2):
                        for term in range(nterm):
                            src = prod8 if term == 0 else plo8
                            stat = src[:, 2 * hp:2 * hp + 2, tsl]
                            st = (hp == 0 and term == 0)
                            sp = (hp == HCH // 2 - 1 and term == nterm - 1)
                            for dh in range(2):
                                nc.tensor.matmul(
                                    pd[dh][:], stat,
                                    wd_s[:, 2 * hp:2 * hp + 2, ts(dh, 512)],
                                    start=st, stop=sp, perf_mode=DR)
                    h2 = h2_p.tile([128, D], BF16, name=f"h2_{tg}_{t4}",
                                   tag="h2")
                    for dh in range(2):
                        dsl = ts(dh, 512)
                        nc.vector.scalar_tensor_tensor(
                            out=h2[:, dsl], in0=pd[dh][:], scalar=INV,
                            in1=x1_list[t4][:, dsl], op0=ALU.mult, op1=ALU.add)
                    nc.gpsimd.tensor_tensor(out=h2[:], in0=h2[:], in1=bdb[:],
                                            op=ALU.add)
                    stats = st_p.tile([128, 2, 6], F32, name=f"s2_{tg}_{t4}",
                                      tag="st2")
                    nc.vector.bn_stats(out=stats[:, 0, :], in_=h2[:, 0:512])
                    nc.vector.bn_stats(out=stats[:, 1, :], in_=h2[:, 512:1024])
                    mv = st_p.tile([128, 2], F32, name=f"mv2_{tg}_{t4}",
                                   tag="mv2")
                    nc.vector.bn_aggr(out=mv[:], in_=stats[:])
                    rstd = st_p.tile([128, 1], F32, name=f"rs2_{tg}_{t4}",
                                     tag="rstd2")
                    nc.scalar.activation(rstd[:], mv[:, 1:2], AF.Sqrt,
                                         bias=epsb[:])
                    nc.vector.reciprocal(out=rstd[:], in_=rstd[:])
                    o_t = out_p.tile([128, D], F32, name=f"o{tg}_{t4}",
                                     tag="ot")
                    nc.vector.tensor_scalar(
                        out=o_t[:], in0=h2[:], scalar1=mv[:, 0:1],
                        scalar2=rstd[:], op0=ALU.subtract, op1=ALU.mult)
                    nc.gpsimd.tensor_tensor(out=o_t[:], in0=o_t[:], in1=g2b[:],
                                            op=ALU.mult)
                    nc.vector.tensor_tensor(out=o_t[:], in0=o_t[:], in1=b2b[:],
                                            op=ALU.add)
                    nc.sync.dma_start(out=out[tok:tok + 128, :], in_=o_t[:])

            attn_block(0)
            for tg in range(NTG):
                gate_up(tg)
                if tg + 1 < NTG:
                    attn_block(tg + 1)
                down_ln2(tg)
                del x1s[tg], x1Ts[tg]

    nc.compile()
    return nc, input_names


# ---------------------------------------------------------------------------
# Host-side wrapper
# ---------------------------------------------------------------------------

B, S, D_MODEL, D_FF = 4, 4096, 1024, 4096
FFN_H = int(2 * D_FF / 3)  # 2730

_cache = {}
LAST_RESULTS = None


def _get_program(T_OWN=2048):
    if T_OWN not in _cache:
        _cache[T_OWN] = build_program(T_OWN)
    return _cache[T_OWN]


def _f8(a):
    return np.clip(np.asarray(a, np.float32), -240.0, 240.0).astype(NP_F8)


def _chunk_pcn(w):
    """[K, N] f32 -> [128, K/128, N] fp8 with K = c*128 + p."""
    K, N = w.shape
    return np.ascontiguousarray(
        _f8(w * WS).reshape(K // 128, 128, N).transpose(1, 0, 2))


def _prep_shared(Wqkv, bqkv, Wg, bg, Wu, bu, Wd, bd, g1, b1, g2, b2):
    f = np.float32
    Wqkv = np.asarray(Wqkv, f)
    sh = {}
    sh["wq8"] = _chunk_pcn(Wqkv[:, 0:1024])
    sh["wk8"] = _chunk_pcn(Wqkv[:, 1024:2048])
    sh["wv8"] = _chunk_pcn(Wqkv[:, 2048:3072])
    bqkv = np.asarray(bqkv, f)
    bq = np.ascontiguousarray(bqkv[0:1024].reshape(DCH, 128).T)
    sh["bq_col"] = bq
    sh["nbq_col"] = np.ascontiguousarray(-bq)
    sh["bk_row"] = bqkv[1024:2048].reshape(1, 1024).astype(NP_BF)
    sh["bv_row"] = bqkv[2048:3072].reshape(1, 1024).astype(NP_BF)
    wg_p = np.zeros((1024, H_PAD), f)
    wg_p[:, :FFN_H] = np.asarray(Wg, f)
    sh["wg8"] = _chunk_pcn(wg_p)
    wu_p = np.zeros((1024, H_PAD), f)
    wu_p[:, :FFN_H] = np.asarray(Wu, f)
    sh["wu8"] = _chunk_pcn(wu_p)
    wd_p = np.zeros((H_PAD, 1024), f)
    wd_p[:FFN_H, :] = np.asarray(Wd, f)
    sh["wd8"] = _chunk_pcn(wd_p)
    bg_p = np.zeros((H_PAD,), f)
    bg_p[:FFN_H] = np.asarray(bg, f)
    sh["bg_col"] = np.ascontiguousarray(bg_p.reshape(HCH, 128).T)
    bu_p = np.zeros((H_PAD,), f)
    bu_p[:FFN_H] = np.asarray(bu, f)
    sh["bu_col"] = np.ascontiguousarray(bu_p.reshape(HCH, 128).T)
    sh["bd_row"] = np.asarray(bd, f).reshape(1, 1024).astype(NP_BF)
    sh["g1_row"] = np.asarray(g1, f).reshape(1, 1024).astype(NP_BF)
    sh["b1_row"] = np.asarray(b1, f).reshape(1, 1024).astype(NP_BF)
    sh["g2_row"] = np.asarray(g2, f).reshape(1, 1024).astype(NP_BF)
    sh["b2_row"] = np.asarray(b2, f).reshape(1, 1024).astype(NP_BF)
    return sh


def make_in_maps(x, Wqkv, bqkv, Wg, bg, Wu, bu, Wd, bd, g1, b1, g2, b2):
    x = np.asarray(x, np.float32)
    sh = _prep_shared(Wqkv, bqkv, Wg, bg, Wu, bu, Wd, bd, g1, b1, g2, b2)
    in_maps = []
    for c in range(8):
        b, h = c // 2, c % 2
        m = dict(sh)
        xo = x[b, h * 2048:(h + 1) * 2048]           # [2048, 1024]
        xT = np.ascontiguousarray(xo.T)              # [1024, 2048]
        x8 = _f8(xT).reshape(DCH, 128, 4, 512)   # [c, p, blk, t']
        m["xT8"] = np.ascontiguousarray(x8.transpose(1, 2, 0, 3))
        m["xtok"] = np.ascontiguousarray(xo.astype(NP_BF))
        in_maps.append(m)
    return in_maps


def kernel(x, Wqkv, bqkv, Wg, bg, Wu, bu, Wd, bd, g1, b1, g2, b2):
    global LAST_RESULTS
    from concourse import bass_utils

    nc, _names = _get_program()
    in_maps = make_in_maps(x, Wqkv, bqkv, Wg, bg, Wu, bu, Wd, bd,
                           g1, b1, g2, b2)
    res = bass_utils.run_bass_kernel_spmd(nc, in_maps, core_ids=list(range(8)))
    LAST_RESULTS = res
    out = np.empty((B, S, D_MODEL), np.float32)
    for c in range(8):
        b, h = c // 2, c % 2
        out[b, h * 2048:(h + 1) * 2048] = res.results[c]["out"]
    return out


# revision 17
# speedup vs baseline: 1.0084x; 1.0084x over previous
"""Trainium2 Bass kernel for nn_Block_54219667145535 (linear-attention block).

fp8 (e4m3) DoubleRow matmuls throughout; weights SBUF-resident; fused
attention+FFN pipeline per 512-token group. 8 cores, 2 per batch (B=4):
each core computes k/v + [D,D] kv state on its own 2048 tokens and
pair-AllReduces (bf16) before attention. Precision recovery: x1 and the
silu-product are fed to the FFN matmuls as two-term fp8 (hi + lo).
"""

import os
import sys
from contextlib import ExitStack

import numpy as np


def _ensure_paths():
    for p in ("/opt/trn_rl_repo", "/root/.axon_site/_ro/trn_rl_repo"):
        if os.path.isdir(p) and p not in sys.path:
            sys.path.insert(0, p)
    try:
        import concourse.bass  # noqa: F401
    except ImportError as e:  # pragma: no cover
        raise ImportError(f"concourse not importable: {e}")


_ensure_paths()

import ml_dtypes  # noqa: E402

import concourse.bass as bass  # noqa: E402
import concourse.bacc as bacc  # noqa: E402
import concourse.tile as tile  # noqa: E402
from concourse import mybir  # noqa: E402
from concourse.masks import make_identity  # noqa: E402

F32 = mybir.dt.float32
BF16 = mybir.dt.bfloat16
F8 = mybir.dt.float8e4
AF = mybir.ActivationFunctionType
ALU = mybir.AluOpType
DR = mybir.MatmulPerfMode.DoubleRow

NP_F8 = ml_dtypes.float8_e4m3  # TRN-compatible e4m3 (max 240)
NP_BF = ml_dtypes.bfloat16

D = 1024
DCH = 8        # d chunks of 128
H_PAD = 2816
HCH = 22       # h chunks of 128
LN_EPS = 1e-5
ATTN_EPS = 1e-6
WS = 64.0      # weight pre-scale (host multiplies W by WS before fp8 cast)
INV = 1.0 / WS
EKV = D + 8    # kv e-columns + 8 extra (col D = ksum via ones-column of v)

X1_2T = True    # two-term fp8 x1 into gate/up
PROD_2T = True  # two-term fp8 silu-product into down


def ts(i, sz):
    return bass.ts(i, sz)


def _bcast_row(row_ap, parts=128):
    """AP reading a [1, N] DRAM row replicated across `parts` partitions."""
    return bass.AP(
        tensor=row_ap.tensor,
        offset=row_ap.offset,
        ap=[[0, parts]] + [list(d) for d in row_ap.ap[1:]],
    )


def build_program(T_OWN=2048, n_cores=8):
    NBLK = T_OWN // 512   # 512-token blocks in P1
    NT4 = T_OWN // 128    # 128-token groups
    NTG = T_OWN // 512    # 512-token groups in P2
    GROUPS = [[c, c + 1] for c in range(0, n_cores, 2)]

    nc = bacc.Bacc(
        "TRN2",
        target_bir_lowering=False,
        debug=False,
        enable_asserts=False,
        num_devices=n_cores,
        num_swdge_queues=4,
    )

    # ---- I/O ----
    xT8 = nc.dram_tensor("xT8", [128, T_OWN // 512, DCH, 512], F8,
                     kind="ExternalInput").ap()
    xtok = nc.dram_tensor("xtok", [T_OWN, D], BF16, kind="ExternalInput").ap()
    wq8 = nc.dram_tensor("wq8", [128, DCH, D], F8, kind="ExternalInput").ap()
    wk8 = nc.dram_tensor("wk8", [128, DCH, D], F8, kind="ExternalInput").ap()
    wv8 = nc.dram_tensor("wv8", [128, DCH, D], F8, kind="ExternalInput").ap()
    wg8 = nc.dram_tensor("wg8", [128, DCH, H_PAD], F8, kind="ExternalInput").ap()
    wu8 = nc.dram_tensor("wu8", [128, DCH, H_PAD], F8, kind="ExternalInput").ap()
    wd8 = nc.dram_tensor("wd8", [128, HCH, D], F8, kind="ExternalInput").ap()
    bq_col = nc.dram_tensor("bq_col", [128, DCH], F32, kind="ExternalInput").ap()
    nbq_col = nc.dram_tensor("nbq_col", [128, DCH], F32, kind="ExternalInput").ap()
    bg_col = nc.dram_tensor("bg_col", [128, HCH], F32, kind="ExternalInput").ap()
    bu_col = nc.dram_tensor("bu_col", [128, HCH], F32, kind="ExternalInput").ap()
    bk_row = nc.dram_tensor("bk_row", [1, D], BF16, kind="ExternalInput").ap()
    bv_row = nc.dram_tensor("bv_row", [1, D], BF16, kind="ExternalInput").ap()
    g1_row = nc.dram_tensor("g1_row", [1, D], BF16, kind="ExternalInput").ap()
    b1_row = nc.dram_tensor("b1_row", [1, D], BF16, kind="ExternalInput").ap()
    g2_row = nc.dram_tensor("g2_row", [1, D], BF16, kind="ExternalInput").ap()
    b2_row = nc.dram_tensor("b2_row", [1, D], BF16, kind="ExternalInput").ap()
    bd_row = nc.dram_tensor("bd_row", [1, D], BF16, kind="ExternalInput").ap()
    out = nc.dram_tensor("out", [T_OWN, D], F32, kind="ExternalOutput").ap()

    input_names = [
        "xT8", "xtok", "wq8", "wk8", "wv8", "wg8", "wu8", "wd8",
        "bq_col", "nbq_col", "bg_col", "bu_col", "bk_row", "bv_row",
        "g1_row", "b1_row", "g2_row", "b2_row", "bd_row",
    ]

    with tile.TileContext(nc) as tc, ExitStack() as top:
        dram = top.enter_context(tc.tile_pool(name="dram", bufs=1, space="DRAM"))
        kvH_ci = [dram.tile([128, DCH // 2, EKV], BF16, name=f"kvH{i}_ci")
                  for i in range(2)]
        kvH_co = [dram.tile([128, DCH // 2, EKV], BF16, name=f"kvH{i}_co")
                  for i in range(2)]

        # pools first, DMA kicks emitted in priority order below
        consts = top.enter_context(tc.tile_pool(name="consts", bufs=1))
        ident_b = consts.tile([128, 128], BF16, name="ident_b")
        epsb = consts.tile([128, 1], F32, name="epsb")
        bq_s = consts.tile([128, DCH], F32, name="bq_s")
        nbq_s = consts.tile([128, DCH], F32, name="nbq_s")
        bg_s = consts.tile([128, HCH], F32, name="bg_s")
        bu_s = consts.tile([128, HCH], F32, name="bu_s")
        bkb = consts.tile([128, D], BF16, name="bkb")
        bvb = consts.tile([128, D], BF16, name="bvb")
        g1b = consts.tile([128, D], BF16, name="g1b")
        b1b = consts.tile([128, D], BF16, name="b1b")
        g2b = consts.tile([128, D], BF16, name="g2b")
        b2b = consts.tile([128, D], BF16, name="b2b")
        bdb = consts.tile([128, D], BF16, name="bdb")

        # FFN weights + long-lived activations (whole-kernel lifetime)
        wpool = top.enter_context(tc.tile_pool(name="wpool", bufs=1))
        wg_s = wpool.tile([128, DCH, H_PAD], F8, name="wg_s")
        wu_s = wpool.tile([128, DCH, H_PAD], F8, name="wu_s")
        wd_s = wpool.tile([128, HCH, D], F8, name="wd_s")
        qp8 = wpool.tile([128, DCH, T_OWN], F8, name="qp8")
        kv8 = wpool.tile([128, DCH, EKV], F8, name="kv8")

        # kv collective staging (SBUF side); closes after the post-collective
        # casts, so it must be opened before the xw pool (LIFO release order)
        kvsb_sc = ExitStack()
        kvsb_p = kvsb_sc.enter_context(tc.tile_pool(name="kvsb", bufs=1))
        kv_sb = kvsb_p.tile([128, DCH, EKV], BF16, name="kv_sb")

        # qkv weights + x / kp / v: freed after the q projection
        xw_sc = ExitStack()
        xw_p = xw_sc.enter_context(tc.tile_pool(name="xw", bufs=1))
        wk_s = xw_p.tile([128, DCH, D], F8, name="wk_s")
        wv_s = xw_p.tile([128, DCH, D], F8, name="wv_s")
        wq_s = xw_p.tile([128, DCH, D], F8, name="wq_s")
        xT8_s = xw_p.tile([128, NBLK, DCH, 512], F8, name="xT8_s")
        kp8 = xw_p.tile([128, NT4, D], F8, name="kp8")
        v8 = xw_p.tile([128, NT4, EKV], F8, name="v8")

        # --- DMA kicks, critical path first -------------------------------
        # k/v weights split by dc-half for fine-grained deps; wk on the
        # scalar ring, wv on the sync ring (parallel with xT8 blk0)
        nc.scalar.dma_start(out=wk_s[:, 0:4, :], in_=wk8[:, 0:4, :])
        nc.sync.dma_start(out=wv_s[:, 0:4, :], in_=wv8[:, 0:4, :])
        nc.scalar.dma_start(out=wk_s[:, 4:8, :], in_=wk8[:, 4:8, :])
        nc.sync.dma_start(out=wv_s[:, 4:8, :], in_=wv8[:, 4:8, :])
        nc.scalar.dma_start(out=bkb[:], in_=_bcast_row(bk_row))
        nc.scalar.dma_start(out=bvb[:], in_=_bcast_row(bv_row))
        nc.scalar.dma_start(out=bq_s[:], in_=bq_col)
        nc.scalar.dma_start(out=nbq_s[:], in_=nbq_col)
        nc.scalar.dma_start(out=wq_s[:], in_=wq8)
        nc.scalar.dma_start(out=bg_s[:], in_=bg_col)
        nc.scalar.dma_start(out=bu_s[:], in_=bu_col)
        nc.scalar.dma_start(out=g1b[:], in_=_bcast_row(g1_row))
        nc.scalar.dma_start(out=b1b[:], in_=_bcast_row(b1_row))
        nc.scalar.dma_start(out=g2b[:], in_=_bcast_row(g2_row))
        nc.scalar.dma_start(out=b2b[:], in_=_bcast_row(b2_row))
        nc.scalar.dma_start(out=bdb[:], in_=_bcast_row(bd_row))
        # device-generated consts
        make_identity(nc, ident_b[:])
        nc.vector.memset(epsb[:], LN_EPS)
        nc.vector.memset(v8[:, :, D:D + 1], 1.0)
        nc.vector.memset(v8[:, :, D + 1:EKV], 0.0)

        # ---------------- P1: k/v projection over own tokens --------------
        with ExitStack() as p1:
            t1_p = p1.enter_context(tc.tile_pool(name="p1tmp", bufs=4))
            ps1 = p1.enter_context(
                tc.tile_pool(name="ps1", bufs=8, space="PSUM"))
            for blk in range(NBLK):
                nc.sync.dma_start(out=xT8_s[:, blk], in_=xT8[:, blk])
                if blk == 1:
                    # FFN weights: kicked once P1 is underway so they don't
                    # compete with the P1-critical loads
                    nc.gpsimd.dma_start(out=wg_s[:], in_=wg8)
                    nc.gpsimd.dma_start(out=wu_s[:], in_=wu8)
                    nc.gpsimd.dma_start(out=wd_s[:], in_=wd8)
                for t4p in range(2):
                    pk = {}
                    pv = {}
                    for li in range(2):
                        t4 = blk * 4 + t4p * 2 + li
                        for h in range(2):
                            pk[(li, h)] = ps1.tile(
                                [128, 512], F32, name=f"pk{t4}_{h}", tag="ps1")
                            pv[(li, h)] = ps1.tile(
                                [128, 512], F32, name=f"pv{t4}_{h}", tag="ps1")
                    for j in range(DCH // 2):
                        for li in range(2):
                            t4 = blk * 4 + t4p * 2 + li
                            lt4 = t4p * 2 + li
                            stat = xT8_s[:, blk, 2 * j:2 * j + 2,
                                         ts(lt4, 128)]
                            st = (j == 0)
                            sp = (j == DCH // 2 - 1)
                            for h in range(2):
                                nc.tensor.matmul(
                                    pk[(li, h)][:], stat,
                                    wk_s[:, 2 * j:2 * j + 2, ts(h, 512)],
                                    start=st, stop=sp, perf_mode=DR)
                                nc.tensor.matmul(
                                    pv[(li, h)][:], stat,
                                    wv_s[:, 2 * j:2 * j + 2, ts(h, 512)],
                                    start=st, stop=sp, perf_mode=DR)
                    for li in range(2):
                        t4 = blk * 4 + t4p * 2 + li
                        for h in range(2):
                            hsl = ts(h, 512)
                            # kp = exp(min(kb,0)) + relu(kb), kb = k + bk
                            kb = t1_p.tile([128, 512], BF16,
                                           name=f"kb{t4}_{h}", tag="kb")
                            nc.vector.scalar_tensor_tensor(
                                out=kb[:], in0=pk[(li, h)][:], scalar=INV,
                                in1=bkb[:, hsl], op0=ALU.mult, op1=ALU.add)
                            rl = t1_p.tile([128, 512], BF16,
                                           name=f"rl{t4}_{h}", tag="rl")
                            nc.scalar.activation(rl[:], kb[:], AF.Relu)
                            nc.gpsimd.tensor_tensor(
                                out=kb[:], in0=kb[:], in1=rl[:],
                                op=ALU.subtract)
                            nc.scalar.activation(kb[:], kb[:], AF.Exp)
                            nc.gpsimd.tensor_tensor(
                                out=kp8[:, t4, hsl], in0=kb[:], in1=rl[:],
                                op=ALU.add)
                            nc.vector.scalar_tensor_tensor(
                                out=v8[:, t4, hsl], in0=pv[(li, h)][:],
                                scalar=INV, in1=bvb[:, hsl],
                                op0=ALU.mult, op1=ALU.add)

        # ---------------- kv sweeps (contract over all own tokens) --------
        with ExitStack() as sw:
            ps_sw = sw.enter_context(
                tc.tile_pool(name="ps_sw", bufs=4, space="PSUM"))
            ps_swc = sw.enter_context(
                tc.tile_pool(name="ps_swc", bufs=2, space="PSUM"))
            for dc in range(DCH):
                dsl = ts(dc, 128)
                pa = ps_sw.tile([128, 512], F32, name=f"pkva{dc}", tag="ps_sw")
                pb = ps_sw.tile([128, 512], F32, name=f"pkvb{dc}", tag="ps_sw")
                pc = ps_swc.tile([128, 8], F32, name=f"pkvc{dc}", tag="ps_swc")
                for tp in range(NT4 // 2):
                    stat = kp8[:, 2 * tp:2 * tp + 2, dsl]
                    st = (tp == 0)
                    sp = (tp == NT4 // 2 - 1)
                    nc.tensor.matmul(pa[:], stat,
                                     v8[:, 2 * tp:2 * tp + 2, 0:512],
                                     start=st, stop=sp, perf_mode=DR)
                    nc.tensor.matmul(pb[:], stat,
                                     v8[:, 2 * tp:2 * tp + 2, 512:1024],
                                     start=st, stop=sp, perf_mode=DR)
                    nc.tensor.matmul(pc[:], stat,
                                     v8[:, 2 * tp:2 * tp + 2, D:EKV],
                                     start=st, stop=sp, perf_mode=DR)
                nc.vector.tensor_copy(out=kv_sb[:, dc, 0:512], in_=pa[:])
                nc.vector.tensor_copy(out=kv_sb[:, dc, 512:1024], in_=pb[:])
                nc.vector.tensor_copy(out=kv_sb[:, dc, D:EKV], in_=pc[:])
                if dc == DCH // 2 - 1 or dc == DCH - 1:
                    # pair-AllReduce of this dc-half, overlapped with the
                    # remaining sweeps / q projection
                    hf = 0 if dc < DCH // 2 else 1
                    hs = slice(hf * (DCH // 2), (hf + 1) * (DCH // 2))
                    nc.sync.dma_start(out=kvH_ci[hf][:], in_=kv_sb[:, hs, :])
                    nc.gpsimd.collective_compute(
                        "AllReduce", ALU.add, ins=[kvH_ci[hf][:]],
                        outs=[kvH_co[hf][:]], replica_groups=GROUPS)

        # ---------------- q projection (overlaps the collectives) ---------
        with ExitStack() as qsc:
            tq_p = qsc.enter_context(tc.tile_pool(name="qtmp", bufs=4))
            ps_q = qsc.enter_context(
                tc.tile_pool(name="ps_q", bufs=4, space="PSUM"))
            for qc in range(DCH):
                pq = [ps_q.tile([128, 512], F32, name=f"pq{qc}_{b}", tag="ps_q")
                      for b in range(NBLK)]
                for j in range(DCH // 2):
                    stat = wq_s[:, 2 * j:2 * j + 2, ts(qc, 128)]
                    st = (j == 0)
                    sp = (j == DCH // 2 - 1)
                    for b in range(NBLK):
                        nc.tensor.matmul(
                            pq[b][:], stat,
                            xT8_s[:, b, 2 * j:2 * j + 2, :],
                            start=st, stop=sp, perf_mode=DR)
                for b in range(NBLK):
                    bsl = ts(b, 512)
                    rn = tq_p.tile([128, 512], BF16, name=f"rn{qc}_{b}",
                                   tag="rn")
                    nc.scalar.activation(rn[:], pq[b][:], AF.Relu,
                                         bias=nbq_s[:, qc:qc + 1], scale=-INV)
                    ex = tq_p.tile([128, 512], BF16, name=f"exq{qc}_{b}",
                                   tag="exq")
                    nc.scalar.activation(ex[:], rn[:], AF.Exp, scale=-1.0)
                    qb = tq_p.tile([128, 512], BF16, name=f"qb{qc}_{b}",
                                   tag="qb")
                    nc.vector.tensor_scalar(
                        out=qb[:], in0=pq[b][:], scalar1=INV,
                        scalar2=bq_s[:, qc:qc + 1], op0=ALU.mult, op1=ALU.add)
                    nc.vector.tensor_tensor(out=rn[:], in0=rn[:], in1=ex[:],
                                            op=ALU.add)
                    nc.vector.tensor_tensor(
                        out=qp8[:, qc, bsl], in0=qb[:], in1=rn[:], op=ALU.add)
        xw_sc.close()

        # ---- collective results -> fp8 (kv8 = kv/64, col D = ksum/64) ----
        with kvsb_sc:
            for hf in range(2):
                hs = slice(hf * (DCH // 2), (hf + 1) * (DCH // 2))
                nc.gpsimd.dma_start(out=kv_sb[:, hs, :], in_=kvH_co[hf][:])
                nc.scalar.activation(kv8[:, hs, :], kv_sb[:, hs, :],
                                     AF.Copy, scale=INV)

        # ---------------- P2: attention + LN1 + FFN + LN2, per 512 toks ---
        with ExitStack() as p2:
            xt_p = p2.enter_context(tc.tile_pool(name="xtok_p", bufs=4))
            h1_p = p2.enter_context(tc.tile_pool(name="h1_p", bufs=3))
            h2_p = p2.enter_context(tc.tile_pool(name="h2_p", bufs=2))
            x1_p = p2.enter_context(tc.tile_pool(name="x1_p", bufs=8))
            x1T_p = p2.enter_context(tc.tile_pool(name="x1T_p", bufs=2))
            pr_p = p2.enter_context(tc.tile_pool(name="pr_p", bufs=1))
            st_p = p2.enter_context(tc.tile_pool(name="st_p", bufs=2))
            den_p = p2.enter_context(tc.tile_pool(name="den_p", bufs=2))
            uv_p = p2.enter_context(tc.tile_pool(name="uv_p", bufs=3))
            out_p = p2.enter_context(tc.tile_pool(name="out_p", bufs=2))
            ps_num = p2.enter_context(
                tc.tile_pool(name="ps_num", bufs=2, space="PSUM"))
            ps_den = p2.enter_context(
                tc.tile_pool(name="ps_den", bufs=1, space="PSUM"))
            ps_tr = p2.enter_context(
                tc.tile_pool(name="ps_tr", bufs=1, space="PSUM"))
            ps_gu = p2.enter_context(
                tc.tile_pool(name="ps_gu", bufs=2, space="PSUM"))
            ps_dn = p2.enter_context(
                tc.tile_pool(name="ps_dn", bufs=2, space="PSUM"))

            prod8 = pr_p.tile([128, HCH, 512], F8, name="prod8")
            plo8 = pr_p.tile([128, HCH, 512], F8, name="plo8") if PROD_2T \
                else None

            x1s = {}    # tg -> [x1 tile per t4]
            x1Ts = {}   # tg -> (hi, lo)

            def attn_block(tg):
                """den + num + h1 + LN1 + transpose for one 512-token group."""
                o = tg * 512
                x1_list = []
                x1s[tg] = x1_list
                pdn = ps_den.tile([128, 4], F32, name=f"pdn{tg}", tag="ps_den")
                rden = den_p.tile([128, 4], F32, name=f"rden{tg}", tag="rden")
                x1T_hi = x1T_p.tile([128, DCH, 512], F8,
                                    name=f"x1h{tg}", tag="x1h")
                x1T_lo = x1T_p.tile([128, DCH, 512], F8,
                                    name=f"x1l{tg}", tag="x1l") if X1_2T \
                    else None
                x1Ts[tg] = (x1T_hi, x1T_lo)

                for t4 in range(4):
                    tok = o + t4 * 128
                    xts = xt_p.tile([128, D], BF16, name=f"xt{tg}_{t4}",
                                    tag="xt")
                    nc.sync.dma_start(out=xts[:], in_=xtok[tok:tok + 128, :])
                    h1 = h1_p.tile([128, D], BF16, name=f"h1_{tg}_{t4}",
                                   tag="h1")
                    pn = {}
                    for ec in range(2):
                        pn[ec] = ps_num.tile([128, 512], F32,
                                             name=f"pn{tg}_{t4}_{ec}",
                                             tag="ps_num")
                    for j in range(DCH // 2):
                        stat = qp8[:, 2 * j:2 * j + 2, tok:tok + 128]
                        st = (j == 0)
                        sp = (j == DCH // 2 - 1)
                        nc.tensor.matmul(
                            pdn[:, t4:t4 + 1], stat,
                            kv8[:, 2 * j:2 * j + 2, D:D + 1],
                            start=st, stop=sp, perf_mode=DR)
                        for ec in range(2):
                            nc.tensor.matmul(
                                pn[ec][:], stat,
                                kv8[:, 2 * j:2 * j + 2, ts(ec, 512)],
                                start=st, stop=sp, perf_mode=DR)
                    nc.vector.tensor_scalar_add(
                        out=rden[:, t4:t4 + 1], in0=pdn[:, t4:t4 + 1],
                        scalar1=ATTN_EPS * INV)
                    nc.vector.reciprocal(out=rden[:, t4:t4 + 1],
                                         in_=rden[:, t4:t4 + 1])
                    for ec in range(2):
                        esl = ts(ec, 512)
                        nc.vector.scalar_tensor_tensor(
                            out=h1[:, esl], in0=pn[ec][:],
                            scalar=rden[:, t4:t4 + 1], in1=xts[:, esl],
                            op0=ALU.mult, op1=ALU.add)
                    # LN1
                    stats = st_p.tile([128, 2, 6], F32, name=f"s1_{tg}_{t4}",
                                      tag="st1")
                    nc.vector.bn_stats(out=stats[:, 0, :], in_=h1[:, 0:512])
                    nc.vector.bn_stats(out=stats[:, 1, :], in_=h1[:, 512:1024])
                    mv = st_p.tile([128, 2], F32, name=f"mv1_{tg}_{t4}",
                                   tag="mv1")
                    nc.vector.bn_aggr(out=mv[:], in_=stats[:])
                    rstd = st_p.tile([128, 1], F32, name=f"rs1_{tg}_{t4}",
                                     tag="rstd1")
                    nc.scalar.activation(rstd[:], mv[:, 1:2], AF.Sqrt,
                                         bias=epsb[:])
                    nc.vector.reciprocal(out=rstd[:], in_=rstd[:])
                    xm = st_p.tile([128, D], BF16, name=f"xm_{tg}_{t4}",
                                   tag="xm")
                    nc.vector.tensor_scalar(
                        out=xm[:], in0=h1[:], scalar1=mv[:, 0:1],
                        scalar2=rstd[:], op0=ALU.subtract, op1=ALU.mult)
                    nc.gpsimd.tensor_tensor(out=xm[:], in0=xm[:], in1=g1b[:],
                                            op=ALU.mult)
                    x1 = x1_p.tile([128, D], BF16, name=f"x1_{tg}_{t4}",
                                   tag="x1")
                    nc.vector.tensor_tensor(out=x1[:], in0=xm[:], in1=b1b[:],
                                            op=ALU.add)
                    x1_list.append(x1)

                    # transpose x1 -> d-major; hi/lo fp8 split
                    ptx = ps_tr.tile([128, DCH, 128], BF16,
                                     name=f"ptx{tg}_{t4}", tag="ps_trx")
                    for dc in range(DCH):
                        nc.tensor.transpose(ptx[:, dc, :],
                                            x1[:, ts(dc, 128)], ident_b[:])
                    tsl = ts(t4, 128)
                    nc.scalar.copy(out=x1T_hi[:, :, tsl], in_=ptx[:])
                    if X1_2T:
                        nc.vector.tensor_tensor(
                            out=x1T_lo[:, :, tsl], in0=ptx[:],
                            in1=x1T_hi[:, :, tsl], op=ALU.subtract)

            def gate_up(tg):
                x1T_hi, x1T_lo = x1Ts[tg]
                for hc in range(HCH):
                    pg = ps_gu.tile([128, 512], F32, name=f"pg{tg}_{hc}",
                                    tag="ps_gu")
                    pu = ps_gu.tile([128, 512], F32, name=f"pu{tg}_{hc}",
                                    tag="ps_gu")
                    nterm = 2 if X1_2T else 1
                    for j in range(DCH // 2):
                        sg = wg_s[:, 2 * j:2 * j + 2, ts(hc, 128)]
                        su = wu_s[:, 2 * j:2 * j + 2, ts(hc, 128)]
                        for term in range(nterm):
                            mv_ = x1T_hi if term == 0 else x1T_lo
                            st = (j == 0 and term == 0)
                            sp = (j == DCH // 2 - 1 and term == nterm - 1)
                            nc.tensor.matmul(
                                pg[:], sg, mv_[:, 2 * j:2 * j + 2, :],
                                start=st, stop=sp, perf_mode=DR)
                            nc.tensor.matmul(
                                pu[:], su, mv_[:, 2 * j:2 * j + 2, :],
                                start=st, stop=sp, perf_mode=DR)
                    sig = uv_p.tile([128, 512], BF16, name=f"sig{tg}_{hc}",
                                    tag="sig")
                    nc.scalar.activation(sig[:], pg[:], AF.Silu,
                                         bias=bg_s[:, hc:hc + 1], scale=INV)
                    uv = uv_p.tile([128, 512], BF16, name=f"uv{tg}_{hc}",
                                   tag="uv")
                    nc.vector.tensor_scalar(
                        out=uv[:], in0=pu[:], scalar1=INV,
                        scalar2=bu_s[:, hc:hc + 1], op0=ALU.mult, op1=ALU.add)
                    if PROD_2T:
                        pt = uv_p.tile([128, 512], BF16, name=f"pt{tg}_{hc}",
                                       tag="pt")
                        nc.gpsimd.tensor_tensor(out=pt[:], in0=uv[:],
                                                in1=sig[:], op=ALU.mult)
                        nc.scalar.copy(out=prod8[:, hc, :], in_=pt[:])
                        nc.vector.tensor_tensor(
                            out=plo8[:, hc, :], in0=pt[:],
                            in1=prod8[:, hc, :], op=ALU.subtract)
                    else:
                        nc.gpsimd.tensor_tensor(out=prod8[:, hc, :],
                                                in0=uv[:], in1=sig[:],
                                                op=ALU.mult)

            def down_ln2(tg):
                o = tg * 512
                x1_list = x1s[tg]
                for t4 in range(4):
                    tok = o + t4 * 128
                    tsl = ts(t4, 128)
                    pd = {}
                    for dh in range(2):
                        pd[dh] = ps_dn.tile([128, 512], F32,
                                            name=f"pd{tg}_{t4}_{dh}",
                                            tag="ps_dn")
                    nterm = 2 if PROD_2T else 1
                    for hp in range(HCH // 2):
                        for term in range(nterm):
                            src = prod8 if term == 0 else plo8
                            stat = src[:, 2 * hp:2 * hp + 2, tsl]
                            st = (hp == 0 and term == 0)
                            sp = (hp == HCH // 2 - 1 and term == nterm - 1)
                            for dh in range(2):
                                nc.tensor.matmul(
                                    pd[dh][:], stat,
                                    wd_s[:, 2 * hp:2 * hp + 2, ts(dh, 512)],
                                    start=st, stop=sp, perf_mode=DR)
                    h2 = h2_p.tile([128, D], BF16, name=f"h2_{tg}_{t4}",
                                   tag="h2")
                    for dh in range(2):
                        dsl = ts(dh, 512)
                        nc.vector.scalar_tensor_tensor(
                            out=h2[:, dsl], in0=pd[dh][:], scalar=INV,
                            in1=x1_list[t4][:, dsl], op0=ALU.mult, op1=ALU.add)
                    nc.gpsimd.tensor_tensor(out=h2[:], in0=h2[:], in1=bdb[:],
                                            op=ALU.add)
                    stats = st_p.tile([128, 2, 6], F32, name=f"s2_{tg}_{t4}",
                                      tag="st2")
                    nc.vector.bn_stats(out=stats[:, 0, :], in_=h2[:, 0:512])
                    nc.vector.bn_stats(out=stats[:, 1, :], in_=h2[:, 512:1024])
                    mv = st_p.tile([128, 2], F32, name=f"mv2_{tg}_{t4}",
                                   tag="mv2")
                    nc.vector.bn_aggr(out=mv[:], in_=stats[:])
                    rstd = st_p.tile([128, 1], F32, name=f"rs2_{tg}_{t4}",
                                     tag="rstd2")
                    nc.scalar.activation(rstd[:], mv[:, 1:2], AF.Sqrt,
                                         bias=epsb[:])
                    nc.vector.reciprocal(out=rstd[:], in_=rstd[:])
                    o_t = out_p.tile([128, D], F32, name=f"o{tg}_{t4}",
                                     tag="ot")
                    nc.vector.tensor_scalar(
                        out=o_t[:], in0=h2[:], scalar1=mv[:, 0:1],
                        scalar2=rstd[:], op0=ALU.subtract, op1=ALU.mult)
                    nc.gpsimd.tensor_tensor(out=o_t[:], in0=o_t[:], in1=g2b[:],
                                            op=ALU.mult)
                    nc.vector.tensor_tensor(out=o_t[:], in0=o_t[:], in1=b2b[:],
                                            op=ALU.add)
                    nc.sync.dma_start(out=out[tok:tok + 128, :], in_=o_t[:])

            attn_block(0)
            for tg in range(NTG):
                gate_up(tg)
                if tg + 1 < NTG:
                    attn_block(tg + 1)
                down_ln2(tg)
                del x1s[tg], x1Ts[tg]

    nc.compile()
    return nc, input_names


# ---------------------------------------------------------------------------
# Host-side wrapper
# ---------------------------------------------------------------------------

B, S, D_MODEL, D_FF = 4, 4096, 1024, 4096
FFN_H = int(2 * D_FF / 3)  # 2730

_cache = {}
LAST_RESULTS = None


def _get_program(T_OWN=2048):
    if T_OWN not in _cache:
        _cache[T_OWN] = build_program(T_OWN)
    return _cache[T_OWN]


def _f8(a):
    return np.clip(np.asarray(a, np.float32), -240.0, 240.0).astype(NP_F8)


def _chunk_pcn(w):
    """[K, N] f32 -> [128, K/128, N] fp8 with K = c*128 + p."""
    K, N = w.shape
    return np.ascontiguousarray(
        _f8(w * WS).reshape(K // 128, 128, N).transpose(1, 0, 2))


def _prep_shared(Wqkv, bqkv, Wg, bg, Wu, bu, Wd, bd, g1, b1, g2, b2):
    f = np.float32
    Wqkv = np.asarray(Wqkv, f)
    sh = {}
    sh["wq8"] = _chunk_pcn(Wqkv[:, 0:1024])
    sh["wk8"] = _chunk_pcn(Wqkv[:, 1024:2048])
    sh["wv8"] = _chunk_pcn(Wqkv[:, 2048:3072])
    bqkv = np.asarray(bqkv, f)
    bq = np.ascontiguousarray(bqkv[0:1024].reshape(DCH, 128).T)
    sh["bq_col"] = bq
    sh["nbq_col"] = np.ascontiguousarray(-bq)
    sh["bk_row"] = bqkv[1024:2048].reshape(1, 1024).astype(NP_BF)
    sh["bv_row"] = bqkv[2048:3072].reshape(1, 1024).astype(NP_BF)
    wg_p = np.zeros((1024, H_PAD), f)
    wg_p[:, :FFN_H] = np.asarray(Wg, f)
    sh["wg8"] = _chunk_pcn(wg_p)
    wu_p = np.zeros((1024, H_PAD), f)
    wu_p[:, :FFN_H] = np.asarray(Wu, f)
    sh["wu8"] = _chunk_pcn(wu_p)
    wd_p = np.zeros((H_PAD, 1024), f)
    wd_p[:FFN_H, :] = np.asarray(Wd, f)
    sh["wd8"] = _chunk_pcn(wd_p)
    bg_p = np.zeros((H_PAD,), f)
    bg_p[:FFN_H] = np.asarray(bg, f)
    sh["bg_col"] = np.ascontiguousarray(bg_p.reshape(HCH, 128).T)
    bu_p = np.zeros((H_PAD,), f)
    bu_p[:FFN_H] = np.asarray(bu, f)
    sh["bu_col"] = np.ascontiguousarray(bu_p.reshape(HCH, 128).T)
    sh["bd_row"] = np.asarray(bd, f).reshape(1, 1024).astype(NP_BF)
    sh["g1_row"] = np.asarray(g1, f).reshape(1, 1024).astype(NP_BF)
    sh["b1_row"] = np.asarray(b1, f).reshape(1, 1024).astype(NP_BF)
    sh["g2_row"] = np.asarray(g2, f).reshape(1, 1024).astype(NP_BF)
    sh["b2_row"] = np.asarray(b2, f).reshape(1, 1024).astype(NP_BF)
    return sh


def make_in_maps(x, Wqkv, bqkv, Wg, bg, Wu, bu, Wd, bd, g1, b1, g2, b2):
    x = np.asarray(x, np.float32)
    sh = _prep_shared(Wqkv, bqkv, Wg, bg, Wu, bu, Wd, bd, g1, b1, g2, b2)
    in_maps = []
    for c in range(8):
        b, h = c // 2, c % 2
        m = dict(sh)
        xo = x[b, h * 2048:(h + 1) * 2048]           # [2048, 1024]
        xT = np.ascontiguousarray(xo.T)              # [1024, 2048]
        x8 = _f8(xT).reshape(DCH, 128, 4, 512)   # [c, p, blk, t']
        m["xT8"] = np.ascontiguousarray(x8.transpose(1, 2, 0, 3))
        m["xtok"] = np.ascontiguousarray(xo.astype(NP_BF))
        in_maps.append(m)
    return in_maps


def kernel(x, Wqkv, bqkv, Wg, bg, Wu, bu, Wd, bd, g1, b1, g2, b2):
    global LAST_RESULTS
    from concourse import bass_utils

    nc, _names = _get_program()
    in_maps = make_in_maps(x, Wqkv, bqkv, Wg, bg, Wu, bu, Wd, bd,
                           g1, b1, g2, b2)
    res = bass_utils.run_bass_kernel_spmd(nc, in_maps, core_ids=list(range(8)))
    LAST_RESULTS = res
    out = np.empty((B, S, D_MODEL), np.float32)
    for c in range(8):
        b, h = c // 2, c % 2
        out[b, h * 2048:(h + 1) * 2048] = res.results[c]["out"]
    return out
